# revision 9
# baseline (speedup 1.0000x reference)
"""Multi-head self-attention Trainium2 kernel (8-core SPMD, no collectives).

Problem: B=4, S=2048, E=1024, H=16, D=64, fp32 I/O.

Sharding: data-parallel over (batch, seq-half): core c handles batch c//2,
query rows [half*1024, half*1024+1024). K/V for the full batch are computed
redundantly by the two cores sharing a batch (cheaper than a collective).

On-chip dataflow (per core), everything in "transposed" space so no on-device
transposes are needed (x is pre-transposed on the host):
  xT [e, s]  --matmul-->  QT [dq, s], KT [dk, s]  (proj outputs transposed)
  xT as lhsT --matmul-->  V  [s, hd]              (natural layout)
  scoresT[k, q]: lhsT=KT_h[d, ktile], rhs=QT_h[d, q]; the two heads of a
    pair sit in partition halves 0:64 / 64:128, so their score matmuls hit
    disjoint PE row groups and can overlap.
  expT = exp(scoresT) on ScalarE (PSUM -> SBUF bf16), one [128,1024] call
    covering both heads of the pair.
  outT_h[d, q] (+ sumexp row 64) = matmul(lhsT=V_aug_h[k, 65], rhs=expT)
    where V_aug has a constant ones column (memset, no matmul needed).
  normalize: sumexp rows are reshaped partition-major via a DRAM bounce so
    one cheap [128,16] reciprocal covers the whole pair, then broadcast
    across the 64 head dims with a K=1 ones matmul, multiply on VectorE.
  out[s, e] = matmul(lhsT=attn_outT[hd, s], rhs=WO[hd, e]) + bO
Q/K biases are added per-partition during PSUM eviction; V/O biases come in
as K=1 ones-row matmuls at the start of each accumulation group.
"""

import os
import sys

import numpy as np

for _p in ("/opt/trn_rl_repo", "/root/.axon_site/_ro/trn_rl_repo"):
    if os.path.isdir(_p) and _p not in sys.path:
        sys.path.append(_p)

import concourse.mybir as mybir
from concourse import bacc
from concourse.bass_utils import run_bass_kernel_spmd
from concourse.tile import TileContext

F16 = mybir.dt.float16
BF16 = mybir.dt.bfloat16
F32 = mybir.dt.float32
EXP = mybir.ActivationFunctionType.Exp

B, S, E = 4, 2048, 1024
H, D = 16, 64
HPAIRS = H // 2        # 8 head pairs (2 heads share a 128-partition block)
SQ = S // 2            # 1024 query rows per core
ET = E // 128          # 8 contraction tiles over embed dim
KTILES = S // 128      # 16 key tiles
N_CORES = 8

_CACHE: dict = {}


def _build():
    nc = bacc.Bacc("TRN2", target_bir_lowering=False)

    xt_d = nc.dram_tensor("xt", [ET, 128, S], F16, kind="ExternalInput")
    wq_d = nc.dram_tensor("wq", [ET, 128, E], F16, kind="ExternalInput")
    wk_d = nc.dram_tensor("wk", [ET, 128, E], F16, kind="ExternalInput")
    wv_d = nc.dram_tensor("wv", [ET, 128, E], F16, kind="ExternalInput")
    wo_d = nc.dram_tensor("wo", [ET, 128, E], F16, kind="ExternalInput")
    bqk_d = nc.dram_tensor("bqk", [128, 2 * ET], F32, kind="ExternalInput")
    brow_d = nc.dram_tensor("brow", [1, 2 * E], F16, kind="ExternalInput")
    out_d = nc.dram_tensor("out", [SQ, E], F32, kind="ExternalOutput")

    with nc.allow_low_precision("intentional fp16/bf16 activations"), TileContext(
        nc
    ) as tc:
        with (
            tc.tile_pool(name="persist", bufs=1) as persist,
            tc.tile_pool(name="qtkt", bufs=2) as qtkt,
            tc.tile_pool(name="work", bufs=2) as work,
            tc.tile_pool(name="dscr", bufs=2, space="DRAM") as dscr,
            tc.tile_pool(name="pbig", bufs=2, space="PSUM") as pbig,
            tc.tile_pool(name="pav", bufs=1, space="PSUM") as pav,
        ):
            # V with a ones column per head: [k%128, ktile, head, 65]
            v_sb = persist.tile([128, KTILES, H, D + 1], BF16, name="v_sb")
            aout_sb = persist.tile([128, ET, SQ], F16, name="aout_sb")
            wo_sb = persist.tile([128, ET, E], F16, name="wo_sb")
            bqk_sb = persist.tile([128, 2 * ET], F32, name="bqk_sb")
            brow_sb = persist.tile([1, 2 * E], F16, name="brow_sb")
            ones_sb = persist.tile([1, 128], F16, name="ones_sb")
            ones_bf = persist.tile([1, 128], BF16, name="ones_bf")
            nc.vector.memset(ones_sb, 1.0)
            nc.vector.memset(ones_bf, 1.0)
            for h in range(H):
                nc.vector.memset(v_sb[:, :, h, D], 1.0)
            nc.sync.dma_start(out=bqk_sb, in_=bqk_d[:, :])
            nc.sync.dma_start(out=brow_sb, in_=brow_d[:, :])

            def big(name):
                return pbig.tile([128, 1024], F32, tag="big", name=name)

            with tc.tile_pool(name="proj", bufs=1) as proj:
                xt_sb = proj.tile([128, ET, S], F16, name="xt_sb")
                wq_sb = proj.tile([128, ET, E], F16, name="wq_sb")
                wk_sb = proj.tile([128, ET, E], F16, name="wk_sb")
                wv_sb = proj.tile([128, ET, E], F16, name="wv_sb")
                for et in range(ET):
                    nc.sync.dma_start(out=xt_sb[:, et, :], in_=xt_d[et, :, :])
                    nc.sync.dma_start(out=wq_sb[:, et, :], in_=wq_d[et, :, :])
                    nc.sync.dma_start(out=wk_sb[:, et, :], in_=wk_d[et, :, :])
                    nc.sync.dma_start(out=wv_sb[:, et, :], in_=wv_d[et, :, :])
                    nc.sync.dma_start(out=wo_sb[:, et, :], in_=wo_d[et, :, :])

                # K=128 contractions are split into hi/lo 64-partition halves
                # targeting two PSUM banks of one tile: disjoint PE row
                # groups stream concurrently (2x), combined at eviction.
                def ksplit_mms(pt, lhs_of, rhs_of, last_stop=True):
                    for et in range(ET):
                        lhs, rhs = lhs_of(et), rhs_of(et)
                        st0, st1 = et == 0, (et == ET - 1) and last_stop
                        nc.tensor.matmul(
                            pt[:, 0:512], lhsT=lhs[0:64], rhs=rhs[0:64],
                            start=st0, stop=st1,
                        )
                        nc.tensor.matmul(
                            pt[:, 512:1024], lhsT=lhs[64:128], rhs=rhs[64:128],
                            start=st0, stop=st1,
                        )

                # ---- V projection: V[s, hd] = x @ WV + bV (bf16 out) ----
                # hi half stops at et==7; bV rides the lo half as a K=1
                # ones matmul; halves are combined during eviction.
                def v_stile2(st):
                    for c in range(2):
                        pv = big(f"pv_{st}_{c}")
                        for et in range(ET):
                            lhs = xt_sb[:, et, st * 128 : (st + 1) * 128]
                            rhs = wv_sb[:, et, c * 512 : (c + 1) * 512]
                            nc.tensor.matmul(
                                pv[:, 0:512], lhsT=lhs[0:64], rhs=rhs[0:64],
                                start=(et == 0), stop=(et == ET - 1),
                            )
                            nc.tensor.matmul(
                                pv[:, 512:1024], lhsT=lhs[64:128],
                                rhs=rhs[64:128],
                                start=(et == 0), stop=False,
                            )
                        nc.tensor.matmul(
                            pv[:, 512:1024],
                            lhsT=ones_sb[0:1, 0:128],
                            rhs=brow_sb[0:1, c * 512 : (c + 1) * 512],
                            start=False, stop=True,
                        )
                        vtmp = work.tile(
                            [128, 512], F32, tag="vtmp", name=f"vtmp_{st}_{c}"
                        )
                        nc.vector.tensor_copy(out=vtmp, in_=pv[:, 0:512])
                        nc.vector.tensor_add(
                            out=v_sb[:, st, c * 8 : (c + 1) * 8, 0:D],
                            in0=pv[:, 512:1024].rearrange(
                                "p (h d) -> p h d", h=8
                            ),
                            in1=vtmp.rearrange("p (h d) -> p h d", h=8),
                        )

                def proj_q(hp):
                    qt_t = qtkt.tile([128, SQ], F16, tag="qt", name=f"qt_{hp}")
                    for q2 in range(2):
                        pq = big(f"pq_{hp}_{q2}")
                        ksplit_mms(
                            pq,
                            lambda et: wq_sb[:, et, hp * 128 : (hp + 1) * 128],
                            lambda et: xt_sb[:, et, q2 * 512 : (q2 + 1) * 512],
                        )
                        dst = qt_t[:, q2 * 512 : (q2 + 1) * 512]
                        nc.vector.tensor_scalar_add(
                            out=dst, in0=pq[:, 0:512],
                            scalar1=bqk_sb[:, hp : hp + 1],
                        )
                        nc.vector.tensor_add(
                            out=dst, in0=pq[:, 512:1024], in1=dst
                        )
                    return qt_t

                def proj_k(hp, kk, kt_t):
                    for q2 in range(2):
                        base = kk * 1024 + q2 * 512
                        pk = big(f"pk_{hp}_{kk}_{q2}")
                        ksplit_mms(
                            pk,
                            lambda et: wk_sb[:, et, hp * 128 : (hp + 1) * 128],
                            lambda et: xt_sb[:, et, base : base + 512],
                        )
                        dst = kt_t[:, base : base + 512]
                        nc.vector.tensor_scalar_add(
                            out=dst, in0=pk[:, 0:512],
                            scalar1=bqk_sb[:, ET + hp : ET + hp + 1],
                        )
                        nc.vector.tensor_add(
                            out=dst, in0=pk[:, 512:1024], in1=dst
                        )

                def project_pair(hp):
                    qt_t = proj_q(hp)
                    kt_t = qtkt.tile([128, S], F16, tag="kt", name=f"kt_{hp}")
                    proj_k(hp, 0, kt_t)
                    proj_k(hp, 1, kt_t)
                    return qt_t, kt_t

                # ---- attention over head pairs ----
                # pair 0: project first, V tiles stream in just-in-time
                cur = project_pair(0)
                v_stile2(0)
                v_stile2(1)
                nxt = {}
                for hp in range(HPAIRS):
                    qt_t, kt_t = cur
                    av = {}
                    for h in range(2):
                        for q2 in range(2):
                            av[(h, q2)] = pav.tile(
                                [65, 512], F32, tag=f"av{h}{q2}",
                                name=f"av_{hp}_{h}_{q2}",
                            )
                    exs = {}
                    for t in range(KTILES):
                        for q2 in range(2):
                            sc = big(f"sc_{hp}_{t}_{q2}")
                            for h in range(2):
                                nc.tensor.matmul(
                                    sc[:, h * 512 : (h + 1) * 512],
                                    lhsT=kt_t[
                                        h * 64 : (h + 1) * 64,
                                        t * 128 : (t + 1) * 128,
                                    ],
                                    rhs=qt_t[
                                        h * 64 : (h + 1) * 64,
                                        q2 * 512 : (q2 + 1) * 512,
                                    ],
                                    start=True, stop=True,
                                )
                            ex = work.tile(
                                [128, 1024], BF16, tag="ex", bufs=4,
                                name=f"ex_{hp}_{t}_{q2}",
                            )
                            nc.scalar.activation(out=ex, in_=sc, func=EXP)
                            exs[q2] = ex
                        # stream remaining V tiles during pair 0's attention
                        if hp == 0 and t + 2 < KTILES:
                            v_stile2(t + 2)
                        # project the next pair mid-loop so its matmuls fill
                        # PE slack instead of stalling ScalarE at the boundary
                        if hp + 1 < HPAIRS:
                            if t == 5:
                                nxt["qt"] = proj_q(hp + 1)
                            elif t == 9:
                                nxt["kt"] = qtkt.tile(
                                    [128, S], F16, tag="kt", name=f"kt_{hp + 1}"
                                )
                                proj_k(hp + 1, 0, nxt["kt"])
                            elif t == 12:
                                proj_k(hp + 1, 1, nxt["kt"])
                        for h in range(2):
                            hg = hp * 2 + h
                            for q2 in range(2):
                                nc.tensor.matmul(
                                    av[(h, q2)],
                                    lhsT=v_sb[:, t, hg, :],
                                    rhs=exs[q2][:, h * 512 : (h + 1) * 512],
                                    start=(t == 0), stop=(t == KTILES - 1),
                                )

                    if hp + 1 < HPAIRS:
                        cur = (nxt["qt"], nxt["kt"])

                    # ---- normalize + park into attn-out ----
                    # gather the 4 sumexp rows (PSUM partition 64) into one
                    # [1, 2048] row, bounce through DRAM into a partition-
                    # major [128, 16] tile for one cheap reciprocal, bounce
                    # back, then K=1 ones-matmul broadcast per head.
                    stage = work.tile(
                        [65, 2048], BF16, tag="stage", bufs=1,
                        name=f"stage_{hp}",
                    )
                    for h in range(2):
                        for q2 in range(2):
                            c = h * 2 + q2
                            nc.vector.tensor_copy(
                                out=stage[64:65, c * 512 : (c + 1) * 512],
                                in_=av[(h, q2)][64:65, :],
                            )
                    scr1 = dscr.tile([2048], BF16, tag="scr1", name=f"scr1_{hp}")
                    scr2 = dscr.tile([2048], BF16, tag="scr2", name=f"scr2_{hp}")
                    rs_t = work.tile([128, 16], BF16, tag="rs", name=f"rs_{hp}")
                    rr_t = work.tile([128, 16], BF16, tag="rr", name=f"rr_{hp}")
                    rrow2 = work.tile(
                        [1, 2048], BF16, tag="rrow2", name=f"rrow2_{hp}"
                    )
                    nc.sync.dma_start(out=scr1[:], in_=stage[64:65, :])
                    nc.sync.dma_start(
                        out=rs_t[:, :], in_=scr1.rearrange("(a b) -> a b", a=128)
                    )
                    nc.vector.reciprocal(out=rr_t, in_=rs_t)
                    nc.sync.dma_start(out=scr2[:], in_=rr_t)
                    nc.sync.dma_start(
                        out=rrow2[0:1, :],
                        in_=scr2.rearrange("(a b) -> a b", a=1),
                    )
                    for h in range(2):
                        rb = big(f"rb_{hp}_{h}")
                        for q2 in range(2):
                            nc.tensor.matmul(
                                rb[0:64, q2 * 512 : (q2 + 1) * 512],
                                lhsT=ones_bf[0:1, 0:64],
                                rhs=rrow2[
                                    0:1, h * 1024 + q2 * 512 : h * 1024 + (q2 + 1) * 512
                                ],
                                start=True, stop=True,
                            )
                        rbc_sb = work.tile(
                            [64, SQ], F32, tag="rbc", name=f"rbc_{hp}_{h}"
                        )
                        nc.vector.tensor_copy(out=rbc_sb, in_=rb[0:64, :])
                        tmp_t = work.tile(
                            [64, SQ], F16, tag="tmp", name=f"tmp_{hp}_{h}"
                        )
                        for q2 in range(2):
                            nc.vector.tensor_mul(
                                out=tmp_t[:, q2 * 512 : (q2 + 1) * 512],
                                in0=av[(h, q2)][0:64, :],
                                in1=rbc_sb[:, q2 * 512 : (q2 + 1) * 512],
                            )
                        nc.sync.dma_start(
                            out=aout_sb[h * 64 : (h + 1) * 64, hp, :], in_=tmp_t
                        )

            # ---- output projection: out[s, e] = attn_out @ WO + bO ----
            # same hi/lo K-split; bO rides the lo half as a K=1 ones matmul
            for st in range(ET):
                for ec in range(2):
                    po = big(f"po_{st}_{ec}")
                    nc.tensor.matmul(
                        po[:, 512:1024],
                        lhsT=ones_sb[0:1, 0:128],
                        rhs=brow_sb[0:1, E + ec * 512 : E + (ec + 1) * 512],
                        start=True, stop=False,
                    )
                    for ht in range(ET):
                        lhs = aout_sb[:, ht, st * 128 : (st + 1) * 128]
                        rhs = wo_sb[:, ht, ec * 512 : (ec + 1) * 512]
                        nc.tensor.matmul(
                            po[:, 0:512], lhsT=lhs[0:64], rhs=rhs[0:64],
                            start=(ht == 0), stop=(ht == ET - 1),
                        )
                        nc.tensor.matmul(
                            po[:, 512:1024], lhsT=lhs[64:128], rhs=rhs[64:128],
                            start=False, stop=(ht == ET - 1),
                        )
                    ot = work.tile([128, 512], F32, tag="ot", name=f"ot_{st}_{ec}")
                    nc.vector.tensor_copy(out=ot, in_=po[:, 0:512])
                    nc.vector.tensor_add(out=ot, in0=po[:, 512:1024], in1=ot)
                    nc.sync.dma_start(
                        out=out_d[
                            st * 128 : (st + 1) * 128, ec * 512 : (ec + 1) * 512
                        ],
                        in_=ot,
                    )

    nc.finalize()
    return nc


def _prep_inputs(x, WQ, bQ, WK, bK, WV, bV, WO, bO):
    f16 = np.float16
    x = np.asarray(x, np.float32)
    WQ = np.asarray(WQ, np.float32)
    WK = np.asarray(WK, np.float32)
    WV = np.asarray(WV, np.float32)
    WO = np.asarray(WO, np.float32)
    bQ = np.asarray(bQ, np.float32)
    bK = np.asarray(bK, np.float32)
    bV = np.asarray(bV, np.float32)
    bO = np.asarray(bO, np.float32)

    wq_np = np.ascontiguousarray(WQ.reshape(ET, 128, E)).astype(f16)
    wk_np = np.ascontiguousarray(WK.reshape(ET, 128, E)).astype(f16)
    wv_np = np.ascontiguousarray(WV.reshape(ET, 128, E)).astype(f16)
    wo_np = np.ascontiguousarray(WO.reshape(ET, 128, E)).astype(f16)

    bqk_np = np.empty((128, 2 * ET), np.float32)
    bqk_np[:, :ET] = bQ.reshape(ET, 128).T
    bqk_np[:, ET:] = bK.reshape(ET, 128).T

    brow_np = np.concatenate([bV, bO]).reshape(1, -1).astype(f16)

    shared = {
        "wq": wq_np, "wk": wk_np, "wv": wv_np, "wo": wo_np,
        "bqk": bqk_np, "brow": brow_np,
    }
    in_maps = []
    for c in range(N_CORES):
        b, half = c // 2, c % 2
        xb = x[b]
        qrows = xb[half * SQ : (half + 1) * SQ]
        orows = xb[(1 - half) * SQ : (2 - half) * SQ]
        # this core's query columns first; attention is permutation-
        # invariant over key order so K/V consistency is preserved
        xt = np.concatenate([qrows.T, orows.T], axis=1)
        xt_np = np.ascontiguousarray(xt.reshape(ET, 128, S)).astype(f16)
        in_maps.append({"xt": xt_np, **shared})
    return in_maps


def kernel(x, WQ, bQ, WK, bK, WV, bV, WO, bO):
    if "nc" not in _CACHE:
        _CACHE["nc"] = _build()
    nc = _CACHE["nc"]
    in_maps = _prep_inputs(x, WQ, bQ, WK, bK, WV, bV, WO, bO)
    res = run_bass_kernel_spmd(nc, in_maps, core_ids=list(range(N_CORES)))
    _CACHE["last_result"] = res
    out = np.empty((B, S, E), np.float32)
    for c, r in enumerate(res.results):
        b, half = c // 2, c % 2
        out[b, half * SQ : (half + 1) * SQ] = r["out"]
    return out


# revision 10
# speedup vs baseline: 1.4540x; 1.4540x over previous
"""Multi-head self-attention Trainium2 kernel (8-core SPMD, no collectives).

Problem: B=4, S=2048, E=1024, H=16, D=64, fp32 I/O.

Sharding: data-parallel over (batch, seq-half): core c handles batch c//2,
query rows [half*1024, half*1024+1024). K/V for the full batch are computed
redundantly by the two cores sharing a batch (cheaper than a collective).

On-chip dataflow (per core), everything in "transposed" space so no on-device
transposes are needed (x is pre-transposed on the host):
  xT [e, s]  --matmul-->  QT [dq, s], KT [dk, s]  (proj outputs transposed)
  xT as lhsT --matmul-->  V  [s, hd]              (natural layout)
  scoresT[k, q]: lhsT=KT_h[d, ktile], rhs=QT_h[d, q]; the two heads of a
    pair sit in partition halves 0:64 / 64:128, so their score matmuls hit
    disjoint PE row groups and stream concurrently.
  expT = exp(scoresT) on ScalarE (PSUM -> SBUF bf16), one [128,1024] call
    covering both heads.
  outT_h[d, q] (+ sumexp in row 64) = matmul(lhsT=V_aug_h[k, 65], rhs=expT)
    where V_aug's ones column is just memset (constant).
  normalize: sumexp rows are reshaped partition-major via a DRAM bounce so
    one cheap [128, 8] reciprocal covers a whole pass, then broadcast
    across the 64 head dims with a K=1 ones matmul, multiply on VectorE.
  out[s, e] = matmul(lhsT=attn_outT[hd, s], rhs=WO[hd, e]) + bO

Each pair's attention runs as two q-passes (512 queries each) so the
attn@V accumulators occupy only 2 PSUM banks, leaving a dedicated 2-bank
slot for projections to run concurrently with attention (PSUM budget:
scores 2x2 + attn@V 2 + proj 2 = 8 banks). Q/K biases are added per-
partition during PSUM eviction; V/O biases ride K=1 ones-row matmuls.
"""

import os
import sys

import numpy as np

for _p in ("/opt/trn_rl_repo", "/root/.axon_site/_ro/trn_rl_repo"):
    if os.path.isdir(_p) and _p not in sys.path:
        sys.path.append(_p)

import concourse.mybir as mybir
from concourse import bacc
from concourse.bass_utils import run_bass_kernel_spmd
from concourse.tile import TileContext

F16 = mybir.dt.float16
BF16 = mybir.dt.bfloat16
F32 = mybir.dt.float32
EXP = mybir.ActivationFunctionType.Exp

B, S, E = 4, 2048, 1024
H, D = 16, 64
HPAIRS = H // 2        # 8 head pairs (2 heads share a 128-partition block)
SQ = S // 2            # 1024 query rows per core
ET = E // 128          # 8 contraction tiles over embed dim
KTILES = S // 128      # 16 key tiles
N_CORES = 8

_CACHE: dict = {}


def _build():
    nc = bacc.Bacc("TRN2", target_bir_lowering=False)

    xt_d = nc.dram_tensor("xt", [ET, 128, S], F16, kind="ExternalInput")
    wq_d = nc.dram_tensor("wq", [ET, 128, E], F16, kind="ExternalInput")
    wk_d = nc.dram_tensor("wk", [ET, 128, E], F16, kind="ExternalInput")
    wv_d = nc.dram_tensor("wv", [ET, 128, E], F16, kind="ExternalInput")
    wo_d = nc.dram_tensor("wo", [ET, 128, E], F16, kind="ExternalInput")
    bqk_d = nc.dram_tensor("bqk", [128, 2 * ET], F32, kind="ExternalInput")
    brow_d = nc.dram_tensor("brow", [1, 2 * E], F16, kind="ExternalInput")
    out_d = nc.dram_tensor("out", [SQ, E], F32, kind="ExternalOutput")

    with nc.allow_low_precision("intentional fp16/bf16 activations"), TileContext(
        nc
    ) as tc:
        with (
            tc.tile_pool(name="persist", bufs=1) as persist,
            tc.tile_pool(name="qtkt", bufs=2) as qtkt,
            tc.tile_pool(name="work", bufs=2) as work,
            tc.tile_pool(name="dscr", bufs=2, space="DRAM") as dscr,
            tc.tile_pool(name="psum", bufs=1, space="PSUM") as psum,
        ):
            # V with a ones column per head: [k%128, ktile, head, 65]
            v_sb = persist.tile([128, KTILES, H, D + 1], BF16, name="v_sb")
            aout_sb = persist.tile([128, ET, SQ], F16, name="aout_sb")
            wo_sb = persist.tile([128, ET, E], F16, name="wo_sb")
            bqk_sb = persist.tile([128, 2 * ET], F32, name="bqk_sb")
            brow_sb = persist.tile([1, 2 * E], F16, name="brow_sb")
            ones_sb = persist.tile([1, 128], F16, name="ones_sb")
            ones_bf = persist.tile([1, 128], BF16, name="ones_bf")
            nc.vector.memset(ones_sb, 1.0)
            nc.vector.memset(ones_bf, 1.0)
            for h in range(H):
                nc.vector.memset(v_sb[:, :, h, D], 1.0)
            nc.sync.dma_start(out=bqk_sb, in_=bqk_d[:, :])
            nc.sync.dma_start(out=brow_sb, in_=brow_d[:, :])

            def sc_tile(name):
                return psum.tile([128, 1024], F32, tag="sc", bufs=2, name=name)

            def pp_tile(name):
                return psum.tile([128, 1024], F32, tag="pp", bufs=1, name=name)

            with tc.tile_pool(name="proj", bufs=1) as proj:
                xt_sb = proj.tile([128, ET, S], F16, name="xt_sb")
                wq_sb = proj.tile([128, ET, E], F16, name="wq_sb")
                wk_sb = proj.tile([128, ET, E], F16, name="wk_sb")
                wv_sb = proj.tile([128, ET, E], F16, name="wv_sb")
                for et in range(ET):
                    nc.sync.dma_start(out=xt_sb[:, et, :], in_=xt_d[et, :, :])
                    nc.sync.dma_start(out=wq_sb[:, et, :], in_=wq_d[et, :, :])
                    nc.sync.dma_start(out=wk_sb[:, et, :], in_=wk_d[et, :, :])
                    nc.sync.dma_start(out=wv_sb[:, et, :], in_=wv_d[et, :, :])
                    nc.sync.dma_start(out=wo_sb[:, et, :], in_=wo_d[et, :, :])

                # ---- V projection for one s-tile (bf16 out + bV) ----
                def v_stile(st):
                    pv = pp_tile(f"pv_{st}")
                    for et in range(ET):
                        lhs = xt_sb[:, et, st * 128 : (st + 1) * 128]
                        for c in range(2):
                            nc.tensor.matmul(
                                pv[:, c * 512 : (c + 1) * 512],
                                lhsT=lhs,
                                rhs=wv_sb[:, et, c * 512 : (c + 1) * 512],
                                start=(et == 0), stop=False,
                            )
                    for c in range(2):
                        nc.tensor.matmul(
                            pv[:, c * 512 : (c + 1) * 512],
                            lhsT=ones_sb[0:1, 0:128],
                            rhs=brow_sb[0:1, c * 512 : (c + 1) * 512],
                            start=False, stop=True,
                        )
                    nc.vector.tensor_copy(
                        out=v_sb[:, st, :, 0:D],
                        in_=pv.rearrange("p (h d) -> p h d", h=H),
                    )

                def proj_q(hp):
                    qt_t = qtkt.tile([128, SQ], F16, tag="qt", name=f"qt_{hp}")
                    pq = pp_tile(f"pq_{hp}")
                    for et in range(ET):
                        for q2 in range(2):
                            nc.tensor.matmul(
                                pq[:, q2 * 512 : (q2 + 1) * 512],
                                lhsT=wq_sb[:, et, hp * 128 : (hp + 1) * 128],
                                rhs=xt_sb[:, et, q2 * 512 : (q2 + 1) * 512],
                                start=(et == 0), stop=(et == ET - 1),
                            )
                    for q2 in range(2):
                        nc.vector.tensor_scalar_add(
                            out=qt_t[:, q2 * 512 : (q2 + 1) * 512],
                            in0=pq[:, q2 * 512 : (q2 + 1) * 512],
                            scalar1=bqk_sb[:, hp : hp + 1],
                        )
                    return qt_t

                def proj_k(hp, kk, kt_t):
                    pk = pp_tile(f"pk_{hp}_{kk}")
                    for et in range(ET):
                        for q2 in range(2):
                            base = kk * 1024 + q2 * 512
                            nc.tensor.matmul(
                                pk[:, q2 * 512 : (q2 + 1) * 512],
                                lhsT=wk_sb[:, et, hp * 128 : (hp + 1) * 128],
                                rhs=xt_sb[:, et, base : base + 512],
                                start=(et == 0), stop=(et == ET - 1),
                            )
                    for q2 in range(2):
                        nc.vector.tensor_scalar_add(
                            out=kt_t[
                                :, kk * 1024 + q2 * 512 : kk * 1024 + (q2 + 1) * 512
                            ],
                            in0=pk[:, q2 * 512 : (q2 + 1) * 512],
                            scalar1=bqk_sb[:, ET + hp : ET + hp + 1],
                        )

                # ---- attention: 8 pairs x 2 q-passes x 16 k-tiles ----
                qt_t = proj_q(0)
                kt_t = qtkt.tile([128, S], F16, tag="kt", name="kt_0")
                proj_k(0, 0, kt_t)
                proj_k(0, 1, kt_t)
                cur = (qt_t, kt_t)
                nxt = {}
                for hp in range(HPAIRS):
                    qt_t, kt_t = cur
                    for q2 in range(2):
                        av = {}
                        for h in range(2):
                            av[h] = psum.tile(
                                [65, 512], F32, tag=f"av{h}", bufs=1,
                                name=f"av_{hp}_{q2}_{h}",
                            )
                        for t in range(KTILES):
                            sc = sc_tile(f"sc_{hp}_{q2}_{t}")
                            for h in range(2):
                                nc.tensor.matmul(
                                    sc[:, h * 512 : (h + 1) * 512],
                                    lhsT=kt_t[
                                        h * 64 : (h + 1) * 64,
                                        t * 128 : (t + 1) * 128,
                                    ],
                                    rhs=qt_t[
                                        h * 64 : (h + 1) * 64,
                                        q2 * 512 : (q2 + 1) * 512,
                                    ],
                                    start=True, stop=True,
                                )
                            ex = work.tile(
                                [128, 1024], BF16, tag="ex", bufs=4,
                                name=f"ex_{hp}_{q2}_{t}",
                            )
                            nc.scalar.activation(out=ex, in_=sc, func=EXP)
                            # pair 0 pass 0: V tiles computed just-in-time
                            if hp == 0 and q2 == 0:
                                v_stile(t)
                            # project the next pair during this pair's
                            # attention using the dedicated proj PSUM slot
                            if hp + 1 < HPAIRS:
                                if q2 == 0 and t == 10:
                                    nxt["qt"] = proj_q(hp + 1)
                                elif q2 == 1 and t == 3:
                                    nxt["kt"] = qtkt.tile(
                                        [128, S], F16, tag="kt",
                                        name=f"kt_{hp + 1}",
                                    )
                                    proj_k(hp + 1, 0, nxt["kt"])
                                elif q2 == 1 and t == 9:
                                    proj_k(hp + 1, 1, nxt["kt"])
                            for h in range(2):
                                nc.tensor.matmul(
                                    av[h],
                                    lhsT=v_sb[:, t, hp * 2 + h, :],
                                    rhs=ex[:, h * 512 : (h + 1) * 512],
                                    start=(t == 0), stop=(t == KTILES - 1),
                                )

                        # ---- normalize this pass; av banks release at the
                        # avcp copy so the next pass can start accumulating.
                        avcp = {}
                        for h in range(2):
                            avcp[h] = work.tile(
                                [65, 512], F32, tag=f"avcp{h}",
                                name=f"avcp_{hp}_{q2}_{h}",
                            )
                            nc.vector.tensor_copy(out=avcp[h], in_=av[h])
                        scr1 = dscr.tile(
                            [2, 512], F32, tag="scr1", name=f"scr1_{hp}_{q2}"
                        )
                        scr2 = dscr.tile(
                            [1024], BF16, tag="scr2", name=f"scr2_{hp}_{q2}"
                        )
                        rs_t = work.tile([128, 8], F32, tag="rs", name=f"rs_{hp}_{q2}")
                        rr_t = work.tile([128, 8], BF16, tag="rr", name=f"rr_{hp}_{q2}")
                        rrow = work.tile(
                            [1, 1024], BF16, tag="rrow", name=f"rrow_{hp}_{q2}"
                        )
                        for h in range(2):
                            nc.sync.dma_start(
                                out=scr1[h, :], in_=avcp[h][64:65, :]
                            )
                        nc.sync.dma_start(
                            out=rs_t[:, :],
                            in_=scr1.rearrange("h (a b) -> (h a) b", a=64),
                        )
                        nc.vector.reciprocal(out=rr_t, in_=rs_t)
                        nc.sync.dma_start(out=scr2[:], in_=rr_t)
                        nc.sync.dma_start(
                            out=rrow[0:1, :],
                            in_=scr2.rearrange("(a b) -> a b", a=1),
                        )
                        rb = pp_tile(f"rb_{hp}_{q2}")
                        for h in range(2):
                            nc.tensor.matmul(
                                rb[0:64, h * 512 : (h + 1) * 512],
                                lhsT=ones_bf[0:1, 0:64],
                                rhs=rrow[0:1, h * 512 : (h + 1) * 512],
                                start=True, stop=True,
                            )
                        rbc_sb = work.tile(
                            [64, 1024], F32, tag="rbc", name=f"rbc_{hp}_{q2}"
                        )
                        nc.vector.tensor_copy(out=rbc_sb, in_=rb[0:64, :])
                        for h in range(2):
                            tmp_t = work.tile(
                                [64, 512], F16, tag=f"tmp{h}",
                                name=f"tmp_{hp}_{q2}_{h}",
                            )
                            nc.vector.tensor_mul(
                                out=tmp_t,
                                in0=avcp[h][0:64, :],
                                in1=rbc_sb[:, h * 512 : (h + 1) * 512],
                            )
                            nc.sync.dma_start(
                                out=aout_sb[
                                    h * 64 : (h + 1) * 64,
                                    hp,
                                    q2 * 512 : (q2 + 1) * 512,
                                ],
                                in_=tmp_t,
                            )
                    if hp + 1 < HPAIRS:
                        cur = (nxt["qt"], nxt["kt"])

            # ---- output projection: out[s, e] = attn_out @ WO + bO ----
            for st in range(ET):
                po = sc_tile(f"po_{st}")
                for ec in range(2):
                    nc.tensor.matmul(
                        po[:, ec * 512 : (ec + 1) * 512],
                        lhsT=ones_sb[0:1, 0:128],
                        rhs=brow_sb[0:1, E + ec * 512 : E + (ec + 1) * 512],
                        start=True, stop=False,
                    )
                    for ht in range(ET):
                        nc.tensor.matmul(
                            po[:, ec * 512 : (ec + 1) * 512],
                            lhsT=aout_sb[:, ht, st * 128 : (st + 1) * 128],
                            rhs=wo_sb[:, ht, ec * 512 : (ec + 1) * 512],
                            start=False, stop=(ht == ET - 1),
                        )
                for ec in range(2):
                    ot = work.tile([128, 512], F32, tag="ot", name=f"ot_{st}_{ec}")
                    nc.vector.tensor_copy(
                        out=ot, in_=po[:, ec * 512 : (ec + 1) * 512]
                    )
                    nc.sync.dma_start(
                        out=out_d[
                            st * 128 : (st + 1) * 128, ec * 512 : (ec + 1) * 512
                        ],
                        in_=ot,
                    )

    nc.finalize()
    return nc


def _prep_inputs(x, WQ, bQ, WK, bK, WV, bV, WO, bO):
    f16 = np.float16
    x = np.asarray(x, np.float32)
    WQ = np.asarray(WQ, np.float32)
    WK = np.asarray(WK, np.float32)
    WV = np.asarray(WV, np.float32)
    WO = np.asarray(WO, np.float32)
    bQ = np.asarray(bQ, np.float32)
    bK = np.asarray(bK, np.float32)
    bV = np.asarray(bV, np.float32)
    bO = np.asarray(bO, np.float32)

    wq_np = np.ascontiguousarray(WQ.reshape(ET, 128, E)).astype(f16)
    wk_np = np.ascontiguousarray(WK.reshape(ET, 128, E)).astype(f16)
    wv_np = np.ascontiguousarray(WV.reshape(ET, 128, E)).astype(f16)
    wo_np = np.ascontiguousarray(WO.reshape(ET, 128, E)).astype(f16)

    bqk_np = np.empty((128, 2 * ET), np.float32)
    bqk_np[:, :ET] = bQ.reshape(ET, 128).T
    bqk_np[:, ET:] = bK.reshape(ET, 128).T

    brow_np = np.concatenate([bV, bO]).reshape(1, -1).astype(f16)

    shared = {
        "wq": wq_np, "wk": wk_np, "wv": wv_np, "wo": wo_np,
        "bqk": bqk_np, "brow": brow_np,
    }
    in_maps = []
    for c in range(N_CORES):
        b, half = c // 2, c % 2
        xb = x[b]
        qrows = xb[half * SQ : (half + 1) * SQ]
        orows = xb[(1 - half) * SQ : (2 - half) * SQ]
        # this core's query columns first; attention is permutation-
        # invariant over key order so K/V consistency is preserved
        xt = np.concatenate([qrows.T, orows.T], axis=1)
        xt_np = np.ascontiguousarray(xt.reshape(ET, 128, S)).astype(f16)
        in_maps.append({"xt": xt_np, **shared})
    return in_maps


def kernel(x, WQ, bQ, WK, bK, WV, bV, WO, bO):
    if "nc" not in _CACHE:
        _CACHE["nc"] = _build()
    nc = _CACHE["nc"]
    in_maps = _prep_inputs(x, WQ, bQ, WK, bK, WV, bV, WO, bO)
    res = run_bass_kernel_spmd(nc, in_maps, core_ids=list(range(N_CORES)))
    _CACHE["last_result"] = res
    out = np.empty((B, S, E), np.float32)
    for c, r in enumerate(res.results):
        b, half = c // 2, c % 2
        out[b, half * SQ : (half + 1) * SQ] = r["out"]
    return out


# revision 13
# speedup vs baseline: 1.5017x; 1.0328x over previous
"""Multi-head self-attention Trainium2 kernel (8-core SPMD, no collectives).

Problem: B=4, S=2048, E=1024, H=16, D=64, fp32 I/O.

Sharding: data-parallel over (batch, seq-half): core c handles batch c//2,
query rows [half*1024, half*1024+1024). K/V for the full batch are computed
redundantly by the two cores sharing a batch (cheaper than a collective).

On-chip dataflow (per core), everything in "transposed" space so no on-device
transposes are needed (x is pre-transposed on the host):
  xT [e, s]  --matmul-->  QT [dq, s], KT [dk, s]  (proj outputs transposed)
  xT as lhsT --matmul-->  V  [s, hd]              (natural layout)
  scoresT[k, q]: lhsT=KT_h[d, ktile], rhs=QT_h[d, q]; the two heads of a
    pair sit in partition halves 0:64 / 64:128, so their score matmuls hit
    disjoint PE row groups and stream concurrently.
  expT = exp(scoresT) on ScalarE (PSUM -> SBUF bf16), one [128,1024] call
    covering both heads.
  outT_h[d, q] (+ sumexp in row 64) = matmul(lhsT=V_aug_h[k, 65], rhs=expT)
    where V_aug's ones column is just memset (constant).
  normalize: sumexp rows are reshaped partition-major via a DRAM bounce so
    one cheap [128, 8] reciprocal covers a whole pass, then broadcast
    across the 64 head dims with a K=1 ones matmul, multiply on VectorE.
  out[s, e] = matmul(lhsT=attn_outT[hd, s], rhs=WO[hd, e]) + bO

Each pair's attention runs as two q-passes (512 queries each) so the
attn@V accumulators occupy only 2 PSUM banks, leaving a dedicated 2-bank
slot for projections to run concurrently with attention (PSUM budget:
scores 2x2 + attn@V 2 + proj 2 = 8 banks). Q/K biases are added per-
partition during PSUM eviction; V/O biases ride K=1 ones-row matmuls.
"""

import os
import sys

import numpy as np

for _p in ("/opt/trn_rl_repo", "/root/.axon_site/_ro/trn_rl_repo"):
    if os.path.isdir(_p) and _p not in sys.path:
        sys.path.append(_p)

import concourse.mybir as mybir
from concourse import bacc
from concourse.bass_utils import run_bass_kernel_spmd
from concourse.tile import TileContext

F16 = mybir.dt.float16
BF16 = mybir.dt.bfloat16
F32 = mybir.dt.float32
EXP = mybir.ActivationFunctionType.Exp

B, S, E = 4, 2048, 1024
H, D = 16, 64
HPAIRS = H // 2        # 8 head pairs (2 heads share a 128-partition block)
SQ = S // 2            # 1024 query rows per core
ET = E // 128          # 8 contraction tiles over embed dim
KTILES = S // 128      # 16 key tiles
N_CORES = 8

_CACHE: dict = {}


def _build():
    nc = bacc.Bacc("TRN2", target_bir_lowering=False)

    xt_d = nc.dram_tensor("xt", [ET, 128, S], F16, kind="ExternalInput")
    wq_d = nc.dram_tensor("wq", [ET, 128, E], F16, kind="ExternalInput")
    wk_d = nc.dram_tensor("wk", [ET, 128, E], F16, kind="ExternalInput")
    wv_d = nc.dram_tensor("wv", [ET, 128, E], F16, kind="ExternalInput")
    wo_d = nc.dram_tensor("wo", [ET, 128, E], F16, kind="ExternalInput")
    bqk_d = nc.dram_tensor("bqk", [128, 2 * ET], F32, kind="ExternalInput")
    brow_d = nc.dram_tensor("brow", [1, 2 * E], F16, kind="ExternalInput")
    out_d = nc.dram_tensor("out", [SQ, E], F32, kind="ExternalOutput")

    with nc.allow_low_precision("intentional fp16/bf16 activations"), TileContext(
        nc
    ) as tc:
        with (
            tc.tile_pool(name="persist", bufs=1) as persist,
            tc.tile_pool(name="qtkt", bufs=2) as qtkt,
            tc.tile_pool(name="work", bufs=2) as work,
            tc.tile_pool(name="dscr", bufs=2, space="DRAM") as dscr,
            tc.tile_pool(name="psum", bufs=1, space="PSUM") as psum,
        ):
            # V with a ones column per head: [k%128, ktile, head, 65]
            v_sb = persist.tile([128, KTILES, H, D + 1], BF16, name="v_sb")
            aout_sb = persist.tile([128, ET, SQ], F16, name="aout_sb")
            wo_sb = persist.tile([128, ET, E], F16, name="wo_sb")
            bqk_sb = persist.tile([128, 2 * ET], F32, name="bqk_sb")
            brow_sb = persist.tile([1, 2 * E], F16, name="brow_sb")
            ones_sb = persist.tile([1, 128], F16, name="ones_sb")
            ones_bf = persist.tile([1, 128], BF16, name="ones_bf")
            nc.vector.memset(ones_sb, 1.0)
            nc.vector.memset(ones_bf, 1.0)
            for h in range(H):
                nc.vector.memset(v_sb[:, :, h, D], 1.0)
            nc.sync.dma_start(out=bqk_sb, in_=bqk_d[:, :])
            nc.sync.dma_start(out=brow_sb, in_=brow_d[:, :])

            def sc_tile(name):
                return psum.tile([128, 1024], F32, tag="sc", bufs=2, name=name)

            def pp_tile(name):
                return psum.tile([128, 1024], F32, tag="pp", bufs=1, name=name)

            with tc.tile_pool(name="proj", bufs=1) as proj:
                xt_sb = proj.tile([128, ET, S], F16, name="xt_sb")
                wq_sb = proj.tile([128, ET, E], F16, name="wq_sb")
                wk_sb = proj.tile([128, ET, E], F16, name="wk_sb")
                wv_sb = proj.tile([128, ET, E], F16, name="wv_sb")
                for et in range(ET):
                    nc.sync.dma_start(out=xt_sb[:, et, :], in_=xt_d[et, :, :])
                    nc.sync.dma_start(out=wq_sb[:, et, :], in_=wq_d[et, :, :])
                    nc.sync.dma_start(out=wk_sb[:, et, :], in_=wk_d[et, :, :])
                    nc.sync.dma_start(out=wv_sb[:, et, :], in_=wv_d[et, :, :])
                    nc.sync.dma_start(out=wo_sb[:, et, :], in_=wo_d[et, :, :])

                # ---- V projection for one s-tile (bf16 out + bV) ----
                # uses the double-buffered sc slots; runs upfront while the
                # pp slot projects pair 0 concurrently
                def v_stile(st):
                    pv = sc_tile(f"pv_{st}")
                    for et in range(ET):
                        lhs = xt_sb[:, et, st * 128 : (st + 1) * 128]
                        for c in range(2):
                            nc.tensor.matmul(
                                pv[:, c * 512 : (c + 1) * 512],
                                lhsT=lhs,
                                rhs=wv_sb[:, et, c * 512 : (c + 1) * 512],
                                start=(et == 0), stop=False,
                            )
                    for c in range(2):
                        nc.tensor.matmul(
                            pv[:, c * 512 : (c + 1) * 512],
                            lhsT=ones_sb[0:1, 0:128],
                            rhs=brow_sb[0:1, c * 512 : (c + 1) * 512],
                            start=False, stop=True,
                        )
                    nc.vector.tensor_copy(
                        out=v_sb[:, st, :, 0:D],
                        in_=pv.rearrange("p (h d) -> p h d", h=H),
                    )

                def proj_q(hp):
                    qt_t = qtkt.tile([128, SQ], F16, tag="qt", name=f"qt_{hp}")
                    pq = pp_tile(f"pq_{hp}")
                    for et in range(ET):
                        for q2 in range(2):
                            nc.tensor.matmul(
                                pq[:, q2 * 512 : (q2 + 1) * 512],
                                lhsT=wq_sb[:, et, hp * 128 : (hp + 1) * 128],
                                rhs=xt_sb[:, et, q2 * 512 : (q2 + 1) * 512],
                                start=(et == 0), stop=(et == ET - 1),
                            )
                    for q2 in range(2):
                        nc.vector.tensor_scalar_add(
                            out=qt_t[:, q2 * 512 : (q2 + 1) * 512],
                            in0=pq[:, q2 * 512 : (q2 + 1) * 512],
                            scalar1=bqk_sb[:, hp : hp + 1],
                        )
                    return qt_t

                def proj_k(hp, kk, kt_t):
                    pk = pp_tile(f"pk_{hp}_{kk}")
                    for et in range(ET):
                        for q2 in range(2):
                            base = kk * 1024 + q2 * 512
                            nc.tensor.matmul(
                                pk[:, q2 * 512 : (q2 + 1) * 512],
                                lhsT=wk_sb[:, et, hp * 128 : (hp + 1) * 128],
                                rhs=xt_sb[:, et, base : base + 512],
                                start=(et == 0), stop=(et == ET - 1),
                            )
                    for q2 in range(2):
                        nc.vector.tensor_scalar_add(
                            out=kt_t[
                                :, kk * 1024 + q2 * 512 : kk * 1024 + (q2 + 1) * 512
                            ],
                            in0=pk[:, q2 * 512 : (q2 + 1) * 512],
                            scalar1=bqk_sb[:, ET + hp : ET + hp + 1],
                        )

                # ---- attention: 8 pairs x 2 q-passes x 16 k-tiles ----
                qt_t = proj_q(0)
                kt_t = qtkt.tile([128, S], F16, tag="kt", name="kt_0")
                proj_k(0, 0, kt_t)
                proj_k(0, 1, kt_t)
                for st in range(KTILES):
                    v_stile(st)
                cur = (qt_t, kt_t)
                nxt = {}
                for hp in range(HPAIRS):
                    qt_t, kt_t = cur
                    for q2 in range(2):
                        av = {}
                        for h in range(2):
                            av[h] = psum.tile(
                                [65, 512], F32, tag=f"av{h}", bufs=1,
                                name=f"av_{hp}_{q2}_{h}",
                            )
                        for t in range(KTILES):
                            sc = sc_tile(f"sc_{hp}_{q2}_{t}")
                            for h in range(2):
                                nc.tensor.matmul(
                                    sc[:, h * 512 : (h + 1) * 512],
                                    lhsT=kt_t[
                                        h * 64 : (h + 1) * 64,
                                        t * 128 : (t + 1) * 128,
                                    ],
                                    rhs=qt_t[
                                        h * 64 : (h + 1) * 64,
                                        q2 * 512 : (q2 + 1) * 512,
                                    ],
                                    start=True, stop=True,
                                )
                            ex = work.tile(
                                [128, 1024], BF16, tag="ex", bufs=6,
                                name=f"ex_{hp}_{q2}_{t}",
                            )
                            nc.scalar.activation(out=ex, in_=sc, func=EXP)
                            # project the next pair during this pair's
                            # attention using the dedicated proj PSUM slot
                            if hp + 1 < HPAIRS:
                                if q2 == 0 and t == 10:
                                    nxt["qt"] = proj_q(hp + 1)
                                elif q2 == 1 and t == 3:
                                    nxt["kt"] = qtkt.tile(
                                        [128, S], F16, tag="kt",
                                        name=f"kt_{hp + 1}",
                                    )
                                    proj_k(hp + 1, 0, nxt["kt"])
                                elif q2 == 1 and t == 9:
                                    proj_k(hp + 1, 1, nxt["kt"])
                            for h in range(2):
                                nc.tensor.matmul(
                                    av[h],
                                    lhsT=v_sb[:, t, hp * 2 + h, :],
                                    rhs=ex[:, h * 512 : (h + 1) * 512],
                                    start=(t == 0), stop=(t == KTILES - 1),
                                )

                        # ---- normalize this pass; av banks release at the
                        # avcp copy so the next pass can start accumulating.
                        avcp = {}
                        for h in range(2):
                            avcp[h] = work.tile(
                                [65, 512], F32, tag=f"avcp{h}",
                                name=f"avcp_{hp}_{q2}_{h}",
                            )
                            nc.vector.tensor_copy(out=avcp[h], in_=av[h])
                        scr1 = dscr.tile(
                            [2, 512], F32, tag="scr1", name=f"scr1_{hp}_{q2}"
                        )
                        scr2 = dscr.tile(
                            [1024], BF16, tag="scr2", name=f"scr2_{hp}_{q2}"
                        )
                        rs_t = work.tile([128, 8], F32, tag="rs", name=f"rs_{hp}_{q2}")
                        rr_t = work.tile([128, 8], BF16, tag="rr", name=f"rr_{hp}_{q2}")
                        rrow = work.tile(
                            [1, 1024], BF16, tag="rrow", name=f"rrow_{hp}_{q2}"
                        )
                        for h in range(2):
                            nc.sync.dma_start(
                                out=scr1[h, :], in_=avcp[h][64:65, :]
                            )
                        nc.sync.dma_start(
                            out=rs_t[:, :],
                            in_=scr1.rearrange("h (a b) -> (h a) b", a=64),
                        )
                        nc.vector.reciprocal(out=rr_t, in_=rs_t)
                        nc.sync.dma_start(out=scr2[:], in_=rr_t)
                        nc.sync.dma_start(
                            out=rrow[0:1, :],
                            in_=scr2.rearrange("(a b) -> a b", a=1),
                        )
                        rb = pp_tile(f"rb_{hp}_{q2}")
                        for h in range(2):
                            nc.tensor.matmul(
                                rb[0:64, h * 512 : (h + 1) * 512],
                                lhsT=ones_bf[0:1, 0:64],
                                rhs=rrow[0:1, h * 512 : (h + 1) * 512],
                                start=True, stop=True,
                            )
                        rbc_sb = work.tile(
                            [64, 1024], F32, tag="rbc", name=f"rbc_{hp}_{q2}"
                        )
                        nc.vector.tensor_copy(out=rbc_sb, in_=rb[0:64, :])
                        for h in range(2):
                            tmp_t = work.tile(
                                [64, 512], F16, tag=f"tmp{h}",
                                name=f"tmp_{hp}_{q2}_{h}",
                            )
                            nc.vector.tensor_mul(
                                out=tmp_t,
                                in0=avcp[h][0:64, :],
                                in1=rbc_sb[:, h * 512 : (h + 1) * 512],
                            )
                            nc.sync.dma_start(
                                out=aout_sb[
                                    h * 64 : (h + 1) * 64,
                                    hp,
                                    q2 * 512 : (q2 + 1) * 512,
                                ],
                                in_=tmp_t,
                            )
                    if hp + 1 < HPAIRS:
                        cur = (nxt["qt"], nxt["kt"])

            # ---- output projection: out[s, e] = attn_out @ WO + bO ----
            for st in range(ET):
                po = sc_tile(f"po_{st}")
                for ec in range(2):
                    nc.tensor.matmul(
                        po[:, ec * 512 : (ec + 1) * 512],
                        lhsT=ones_sb[0:1, 0:128],
                        rhs=brow_sb[0:1, E + ec * 512 : E + (ec + 1) * 512],
                        start=True, stop=False,
                    )
                    for ht in range(ET):
                        nc.tensor.matmul(
                            po[:, ec * 512 : (ec + 1) * 512],
                            lhsT=aout_sb[:, ht, st * 128 : (st + 1) * 128],
                            rhs=wo_sb[:, ht, ec * 512 : (ec + 1) * 512],
                            start=False, stop=(ht == ET - 1),
                        )
                for ec in range(2):
                    ot = work.tile([128, 512], F32, tag="ot", name=f"ot_{st}_{ec}")
                    nc.vector.tensor_copy(
                        out=ot, in_=po[:, ec * 512 : (ec + 1) * 512]
                    )
                    nc.sync.dma_start(
                        out=out_d[
                            st * 128 : (st + 1) * 128, ec * 512 : (ec + 1) * 512
                        ],
                        in_=ot,
                    )

    nc.finalize()
    return nc


def _prep_inputs(x, WQ, bQ, WK, bK, WV, bV, WO, bO):
    f16 = np.float16
    x = np.asarray(x, np.float32)
    WQ = np.asarray(WQ, np.float32)
    WK = np.asarray(WK, np.float32)
    WV = np.asarray(WV, np.float32)
    WO = np.asarray(WO, np.float32)
    bQ = np.asarray(bQ, np.float32)
    bK = np.asarray(bK, np.float32)
    bV = np.asarray(bV, np.float32)
    bO = np.asarray(bO, np.float32)

    wq_np = np.ascontiguousarray(WQ.reshape(ET, 128, E)).astype(f16)
    wk_np = np.ascontiguousarray(WK.reshape(ET, 128, E)).astype(f16)
    wv_np = np.ascontiguousarray(WV.reshape(ET, 128, E)).astype(f16)
    wo_np = np.ascontiguousarray(WO.reshape(ET, 128, E)).astype(f16)

    bqk_np = np.empty((128, 2 * ET), np.float32)
    bqk_np[:, :ET] = bQ.reshape(ET, 128).T
    bqk_np[:, ET:] = bK.reshape(ET, 128).T

    brow_np = np.concatenate([bV, bO]).reshape(1, -1).astype(f16)

    shared = {
        "wq": wq_np, "wk": wk_np, "wv": wv_np, "wo": wo_np,
        "bqk": bqk_np, "brow": brow_np,
    }
    in_maps = []
    for c in range(N_CORES):
        b, half = c // 2, c % 2
        xb = x[b]
        qrows = xb[half * SQ : (half + 1) * SQ]
        orows = xb[(1 - half) * SQ : (2 - half) * SQ]
        # this core's query columns first; attention is permutation-
        # invariant over key order so K/V consistency is preserved
        xt = np.concatenate([qrows.T, orows.T], axis=1)
        xt_np = np.ascontiguousarray(xt.reshape(ET, 128, S)).astype(f16)
        in_maps.append({"xt": xt_np, **shared})
    return in_maps


def kernel(x, WQ, bQ, WK, bK, WV, bV, WO, bO):
    if "nc" not in _CACHE:
        _CACHE["nc"] = _build()
    nc = _CACHE["nc"]
    in_maps = _prep_inputs(x, WQ, bQ, WK, bK, WV, bV, WO, bO)
    res = run_bass_kernel_spmd(nc, in_maps, core_ids=list(range(N_CORES)))
    _CACHE["last_result"] = res
    out = np.empty((B, S, E), np.float32)
    for c, r in enumerate(res.results):
        b, half = c // 2, c % 2
        out[b, half * SQ : (half + 1) * SQ] = r["out"]
    return out
